# revision 18
# baseline (speedup 1.0000x reference)
"""Trainium2 Bass kernel for nn_Agent_50500225466537 (retrieval_knn GCN agent).

Strategy (8-core SPMD, 1D row-shard of the N=8192 node dim, ZERO collectives):
  - Host prep computes everything that is O(N^2) or smaller and data-layout
    shaped: the GCN degrees d = colsum(A+I), the scaled layer-1 features
    Md = (X @ W1) * 64/d  (fp8, DoubleRow pair-tiled), and the pre-tiled
    (A+I)^T shards (partition-major int8->fp8 via LUT) so every DMA slice
    is one contiguous read per partition.
  - Device, per graph (y then x), each core fully independent, does the
    dominant O(N^2 D) work:
      S^T = Md^T @ AhT                fp8 DoubleRow matmuls
      h^T = sigmoid(S^T / (64 d_i) + b1) -> bf16 -> DMA out
    Layout: ALL input data is SBUF-resident (at 16MB + md 4MB fit in the
    26MB usable SBUF), DMA'd as ~30 consumption-ordered slices into
    subranges of four big tiles with subtile dependency tracking -- no
    buffer recycling, so the sync-queue input stream free-runs at the
    ~420GB/s HBM read rate while the PE streams DoubleRow matmuls at the
    216ns/512-col peak.  Junk matmuls bridge the ~10us DMA ramp (and are
    interleaved into the first k-blocks) so the HAM clock-gate warms once
    and stays warm.  Each (graph, nh, ih) output quadrant owns one of the
    8 PSUM banks, so epilogue reads never block accumulation (the WAR
    hazard is tile-granular).  The final k-blocks are emitted
    quadrant-major with a per-quadrant mul+sigmoid+DMA-out epilogue; the
    last x chunks are 2/1/1 k-blocks and the very last quadrant's
    epilogue is split in halves across both HWDGE queues, shrinking the
    post-last-DMA tail.  The DMA-completion critical path is set by the
    slowest of the 16 SDMA engines (~1.3MB each at ~26GB/s).
  - Host tail: u = h @ W2 per core, then layer 2 collapses to matvecs
    because W2 is (256, 1): G = sigmoid(((A+I) @ (u/d))/d + b2).  G_y via
    one BLAS matvec, g_x via one row-dot, then the cosine top-11 + softmax
    exactly as the reference.
"""
import os
import sys

for _p in ("/opt/trn_rl_repo", "/root/.axon_site/_ro/trn_rl_repo"):
    if os.path.isdir(_p) and _p not in sys.path:
        sys.path.insert(0, _p)

import numpy as np

import concourse.bacc as bacc
from concourse import bass_utils, mybir, tile

N = 8192
NCORES = 8
R = N // NCORES          # rows per core: 1024
PB = 128                 # partition block
KB = N // PB             # 64 k-blocks
KB2 = KB // 2            # 32 k-block pairs (fp8 DoubleRow)
D = 256                  # feature dim (= hidden dim)
EPS = 1e-8
K_OPP = 11
MDS = 64.0               # fp8 scale for Md (power of two, exact)
NWARM = 10               # junk matmuls to pre-warm the PE clock

F32 = mybir.dt.float32
BF16 = mybir.dt.bfloat16
FP8 = mybir.dt.float8e4
AF = mybir.ActivationFunctionType
DR = mybir.MatmulPerfMode.DoubleRow

class _G:
    """Per-graph emission state."""
    pass


_CACHED_NC = None


def _build_program():
    global _CACHED_NC
    if _CACHED_NC is not None:
        return _CACHED_NC
    nc = bacc.Bacc("TRN2", target_bir_lowering=False, debug=False,
                   enable_asserts=False, num_devices=NCORES)

    gy = _G()
    gx = _G()
    gy.tag, gx.tag = "y", "x"
    gx.ahT = nc.dram_tensor("ahT_x", [PB, KB2, 2, R], FP8,
                            kind="ExternalInput").ap()
    gy.ahT = nc.dram_tensor("ahT_y", [PB, KB2, 2, R], FP8,
                            kind="ExternalInput").ap()
    gx.mdq = nc.dram_tensor("mdq_x", [PB, KB2, 2, D], FP8,
                            kind="ExternalInput").ap()
    gy.mdq = nc.dram_tensor("mdq_y", [PB, KB2, 2, D], FP8,
                            kind="ExternalInput").ap()
    smf_in = nc.dram_tensor("smf32", [PB, 2], F32, kind="ExternalInput").ap()
    smb_in = nc.dram_tensor("smbf", [PB, 2 * R], BF16,
                            kind="ExternalInput").ap()

    gx.hT_out = nc.dram_tensor("hT_x", [PB, 2, R], BF16,
                               kind="ExternalOutput").ap()
    gy.hT_out = nc.dram_tensor("hT_y", [PB, 2, R], BF16,
                               kind="ExternalOutput").ap()

    with tile.TileContext(nc) as tc:
        import contextlib
        with contextlib.ExitStack() as st:
            big = st.enter_context(tc.tile_pool(name="big", bufs=1))
            ps = st.enter_context(
                tc.tile_pool(name="ps", bufs=8, space="PSUM"))

            # SBUF-resident tensors (one slot per tag, no recycling).
            gy.at = big.tile([PB, KB2, 2, R], FP8, tag="at_y", name="at_y")
            gx.at = big.tile([PB, KB2, 2, R], FP8, tag="at_x", name="at_x")
            gy.md = big.tile([PB, KB2, 2, D], FP8, tag="md_y", name="md_y")
            gx.md = big.tile([PB, KB2, 2, D], FP8, tag="md_x", name="md_x")
            smf = big.tile([PB, 2], F32, tag="smf", name="smf")
            smb = big.tile([PB, 2 * R], BF16, tag="smb", name="smb")
            gy.hT = big.tile([PB, 2, R], BF16, tag="hT_y", name="hT_y")
            gx.hT = big.tile([PB, 2, R], BF16, tag="hT_x", name="hT_x")
            wu = big.tile([PB, 512], FP8, tag="wu", name="wu")
            sigp = big.tile([1, 32], F32, tag="sigp", name="sigp")

            gy.rb = smb[:, 0:R]
            gx.rb = smb[:, R:2 * R]

            # ACT sigmoid table preload + PE warmup source (first vector op).
            nc.vector.memset(wu[:], 1.0)
            nc.scalar.activation(sigp[:], wu[0:1, 0:32], AF.Sigmoid)

            # All input DMAs ride the sync HWDGE queue (the scalar HWDGE
            # queue measured ~3x slower to start and stream), in
            # consumption order with a fine-grained head; every slice
            # goes to its own SBUF subrange so the queue never waits.
            stream = [
                (gy, 'md', 0, 2), (gy, 'at', 0, 1), (gy, 'at', 1, 2),
                (gy, 'md', 2, 4), (gy, 'at', 2, 4), (gy, 'md', 4, 8),
                (gy, 'at', 4, 8),
                ('smf',), (gy, 'md', 8, 16), (gy, 'at', 8, 12),
                (gy, 'at', 12, 16),
                ('smb',), (gy, 'md', 16, 24), (gy, 'at', 16, 20),
                (gy, 'at', 20, 24), (gy, 'md', 24, 32), (gy, 'at', 24, 28),
                (gy, 'at', 28, 32),
                (gx, 'md', 0, 8), (gx, 'at', 0, 4), (gx, 'at', 4, 8),
                (gx, 'md', 8, 16), (gx, 'at', 8, 12), (gx, 'at', 12, 16),
                (gx, 'md', 16, 24), (gx, 'at', 16, 20), (gx, 'at', 20, 24),
                (gx, 'md', 24, 32), (gx, 'at', 24, 28), (gx, 'at', 28, 30),
                (gx, 'at', 30, 31), (gx, 'at', 31, 32),
            ]
            for it in stream:
                if it == ('smf',):
                    nc.sync.dma_start(smf[:], smf_in)
                elif it == ('smb',):
                    nc.sync.dma_start(smb[:], smb_in)
                else:
                    g, kind, a, b = it
                    if kind == 'at':
                        nc.sync.dma_start(g.at[:, a:b], g.ahT[:, a:b])
                    else:
                        nc.sync.dma_start(g.md[:, a:b], g.mdq[:, a:b])

            # Eight single-bank PSUM tiles in the 8 pool slots: each
            # (graph, nh, ih) quadrant is its own tile, so an epilogue's
            # in-place mul/sigmoid read of one quadrant never blocks the
            # accumulating matmuls of another (the WAR hazard is tracked
            # at tile granularity).
            gy.psS = [[ps.tile([PB, 512], F32, tag="psS",
                               name=f"psS_y{nh}{ih}") for ih in range(2)]
                      for nh in range(2)]
            gx.psS = [[ps.tile([PB, 512], F32, tag="psS",
                               name=f"psS_x{nh}{ih}") for ih in range(2)]
                      for nh in range(2)]

            # PE pre-warm into psS_y00: junk matmuls bridge the DMA
            # lead-in (the first real matmul's start=True resets the
            # bank); the last two self-pace on the first real tiles.
            psw = gy.psS[0][0]
            for _ in range(NWARM):
                nc.tensor.matmul(psw[:], wu[:, 0:128], wu[:],
                                 start=True, stop=True)
            nc.tensor.matmul(psw[:], wu[:, 0:128],
                             gy.md[:, 0, :, 0:D], start=True, stop=True)
            nc.tensor.matmul(psw[:], wu[:, 0:128],
                             gy.at[:, 0, :, 0:D], start=True, stop=True)

            # Junk filler inserted into graph y's head: the DMA stream
            # takes ~10us to reach line rate, so without filler the PE
            # idles at the early chunk waits and the HAM clock-gate
            # re-throttles.  Filler targets psS_x11, whose first real
            # matmul resets the bank via start=True.
            FILLER = {0: 3, 1: 3, 2: 2, 3: 2, 4: 1, 5: 1}

            def emit_graph(g, filler=None, tail=4):
                psS = g.psS
                junk_tgt = gx.psS[1][1]
                for kb2 in range(KB2 - tail):
                    for nh in range(2):
                        for ih in range(2):
                            nc.tensor.matmul(
                                psS[nh][ih][:],
                                g.md[:, kb2, :, nh * PB:(nh + 1) * PB],
                                g.at[:, kb2, :, ih * 512:(ih + 1) * 512],
                                start=(kb2 == 0), stop=False, perf_mode=DR)
                    if filler:
                        for _ in range(filler.get(kb2, 0)):
                            nc.tensor.matmul(junk_tgt[:], wu[:, 0:128],
                                             wu[:], start=True, stop=True)
                # final k-blocks quadrant-major so each PSUM bank's
                # epilogue starts while the next bank still accumulates
                for nh in range(2):
                    for ih in range(2):
                        ihs = slice(ih * 512, (ih + 1) * 512)
                        p = psS[nh][ih]
                        for kb2 in range(KB2 - tail, KB2):
                            nc.tensor.matmul(
                                p[:],
                                g.md[:, kb2, :, nh * PB:(nh + 1) * PB],
                                g.at[:, kb2, :, ihs],
                                start=False, stop=(kb2 == KB2 - 1),
                                perf_mode=DR)
                        last = (g is gx and nh == 1 and ih == 1)
                        if not last:
                            nc.vector.tensor_mul(p[:], p[:], g.rb[:, ihs])
                            nc.scalar.activation(g.hT[:, nh, ihs], p[:],
                                                 AF.Sigmoid,
                                                 bias=smf[:, nh:nh + 1])
                            nc.sync.dma_start(g.hT_out[:, nh, ihs],
                                              g.hT[:, nh, ihs])
                        else:
                            # split the very last quadrant's epilogue in
                            # two halves so mul/sigmoid pipeline and the
                            # two output-DMA issue latencies overlap on
                            # different HWDGE queues
                            for hh in range(2):
                                c0 = ih * 512 + hh * 256
                                hs = slice(c0, c0 + 256)
                                ph = p[:, hh * 256:(hh + 1) * 256]
                                nc.vector.tensor_mul(ph, ph, g.rb[:, hs])
                                nc.scalar.activation(g.hT[:, nh, hs], ph,
                                                     AF.Sigmoid,
                                                     bias=smf[:, nh:nh + 1])
                                eng = nc.sync if hh == 0 else nc.scalar
                                eng.dma_start(g.hT_out[:, nh, hs],
                                              g.hT[:, nh, hs])

            emit_graph(gy, FILLER, tail=4)
            emit_graph(gx, tail=2)

    nc.compile()
    _CACHED_NC = nc
    return nc


def _prep_in_maps(A_x, A_y, first_embeddings, second_embeddings, W1, b1, W2,
                  b2):
    import ml_dtypes

    # fp8 bit patterns for the exact small ints {0, 1, 2}
    lut = np.array([0.0, 1.0, 2.0], dtype=np.float32).astype(
        ml_dtypes.float8_e4m3fn).view(np.uint8)

    def prep_graph(A, X):
        d = (A.sum(axis=0, dtype=np.int64) + 1).astype(np.float32)
        A8 = A.astype(np.int8)
        A8[np.arange(N), np.arange(N)] += 1
        AT = np.ascontiguousarray(A8.T)  # AT[k, i] = (A+I)[i, k]
        shards = []
        for c in range(NCORES):
            blk = AT[:, c * R:(c + 1) * R].reshape(KB2, 2, PB, R)
            blk = np.ascontiguousarray(blk.transpose(2, 0, 1, 3))
            shards.append(lut[blk].view(ml_dtypes.float8_e4m3fn))
        # scaled layer-1 features, fp8 DoubleRow pair-tiled:
        # mdq[p, kb2, ko, h] = Md[kb2*256 + ko*128 + p, h] * 64/d_k
        Md = (X @ W1) * (np.float32(MDS) / d)[:, None]
        mdq = np.ascontiguousarray(
            Md.reshape(KB2, 2, PB, D).transpose(2, 0, 1, 3)).astype(
                ml_dtypes.float8_e4m3fn)
        return d, shards, mdq

    d_x, shx, mdq_x = prep_graph(A_x, first_embeddings)
    d_y, shy, mdq_y = prep_graph(A_y, second_embeddings)

    smf32 = np.ascontiguousarray(b1.reshape(2, PB).T)

    rb_x = (np.float32(1.0) / (np.float32(MDS) * d_x))
    rb_y = (np.float32(1.0) / (np.float32(MDS) * d_y))
    smbf_list = []
    for c in range(NCORES):
        s = np.empty((PB, 2 * R), dtype=np.float32)
        s[:, 0:R] = rb_y[c * R:(c + 1) * R][None, :]
        s[:, R:2 * R] = rb_x[c * R:(c + 1) * R][None, :]
        smbf_list.append(s.astype(ml_dtypes.bfloat16))

    in_maps = [
        dict(ahT_x=shx[c], ahT_y=shy[c], mdq_x=mdq_x, mdq_y=mdq_y,
             smf32=smf32, smbf=smbf_list[c])
        for c in range(NCORES)
    ]
    return in_maps, d_x, d_y


def _sigmoid(x):
    return 1.0 / (1.0 + np.exp(-x))


def kernel(A_x, A_y, first_embeddings, second_embeddings, W1, b1, W2, b2,
           W_h, W_f, W_p, bias_h, index_x, index_y):
    A_x = np.asarray(A_x)
    A_y = np.asarray(A_y)
    first_embeddings = np.asarray(first_embeddings, dtype=np.float32)
    second_embeddings = np.asarray(second_embeddings, dtype=np.float32)
    W1 = np.asarray(W1, dtype=np.float32)
    b1 = np.asarray(b1, dtype=np.float32)
    W2 = np.asarray(W2, dtype=np.float32)
    b2 = np.asarray(b2, dtype=np.float32)
    W_h = np.asarray(W_h, dtype=np.float32)
    W_f = np.asarray(W_f, dtype=np.float32)
    W_p = np.asarray(W_p, dtype=np.float32)
    bias_h = np.asarray(bias_h, dtype=np.float32)
    ix = int(index_x)
    iy = int(index_y)

    nc = _build_program()
    in_maps, d_x, d_y = _prep_in_maps(A_x, A_y, first_embeddings,
                                      second_embeddings, W1, b1, W2, b2)
    res = bass_utils.run_bass_kernel_spmd(nc, in_maps,
                                          core_ids=list(range(NCORES)))
    results = res.results

    W2_2 = W2[:, 0].reshape(2, PB).T.astype(np.float32)  # [PB, 2]

    def u_full(key):
        return np.concatenate([
            np.einsum("pki,pk->i",
                      np.asarray(results[c][key], dtype=np.float32), W2_2)
            for c in range(NCORES)])

    u_x = u_full("hT_x")
    u_y = u_full("hT_y")

    # ---- host tail (O(N^2) matvec + O(N) ops), fp32 like the reference ----
    row = A_x[ix].astype(np.float32)
    row[ix] += 1.0
    pre = np.float32(row @ (u_x / d_x)) / d_x[ix] + b2[0]
    g_x = _sigmoid(np.float32(pre))

    s = u_y / d_y
    w = A_y.astype(np.float32) @ s + s      # (A_y + I) @ s
    G_y_full = _sigmoid(w / d_y + b2[0]).astype(np.float32)
    g_y = G_y_full[iy]

    cat = np.array([[g_x], [g_y]], dtype=np.float32)        # (2, 1)
    h = _sigmoid(W_h @ cat + bias_h)                        # (1, 1)
    f = np.exp(g_x * W_f * g_y)                             # (1, 1)

    # cosine-similarity top-k over G_y (C = 1)
    num = G_y_full * g_y
    ng = np.maximum(np.abs(G_y_full), np.float32(EPS))
    nv = np.maximum(np.abs(g_y), np.float32(EPS))
    sims = num / (ng * nv)
    idx = np.argsort(-sims, kind="stable")[:K_OPP]
    opp = G_y_full[idx]
    f_oppo = np.float32(np.sum(np.exp(g_x * W_f[0, 0] * opp)))

    I_val = f / f_oppo                                      # (1, 1)
    z = W_p @ np.concatenate([h, I_val], axis=1)            # (1, 2)
    zs = z - z.max(axis=1, keepdims=True)
    ez = np.exp(zs)
    policy = ez / ez.sum(axis=1, keepdims=True)
    return policy.astype(np.float32)
